# revision 1
# baseline (speedup 1.0000x reference)
"""Joint Maximum Mean Discrepancy loss on 8 Trainium2 NeuronCores.

Math: for streams (s0,t0) and (s1,t1), the reference builds per-stream
Gaussian kernels K_r = exp(-gamma_r * dist_r) over feats_r = [src; tgt]
(N=8192 rows), takes their elementwise product, and returns
mean(s2s + t2t - 2*s2t) over the B x B blocks.

Device decomposition:
  exponent E_ij = 2*(W @ W.T)_ij - c_i - c_j
  with W = [sqrt(g0)*X0, sqrt(g1)*X1] (N x 320), c_i = g0*|X0_i|^2 +
  g1*|X1_i|^2, and gamma_r from the closed form
  sum(dist_r) = 2*N*sum(sq_r) - 2*||colsum(X_r)||^2. The joint kernel is
  exp(E) in a single matmul + exp; -c_i and -c_j ride as two extra
  contraction rows (K = 322). Symmetry of E halves the work via a
  block-cyclic cover: core k owns row-chunks {k, k+8} (chunk = 512 rows)
  and computes 17 [512 x 512] blocks — column offsets d=0..8 from row
  chunk k, d=0..7 from row chunk k+8 — counting every unordered
  off-diagonal chunk pair exactly once (weight 2) and diagonals once
  (weight 1). Per-block sums (fp32, one per PSUM partition) return to the
  host, which applies weights/signs and the final reduction in float64.

Per-core device program (SPMD — identical instructions, data differs):
  - lhs  [2, 322, 512]  : [2*W_rows.T ; ones ; -c_rows] for chunks k, k+8
  - rhs  [16, 322, 512] : [W_cols.T ; -c_cols ; ones], chunk-major with
    chunk order rotated by k so the program's chunk index d is core-local
  - 17 blocks x 4 m-tiles: 3 matmuls (K chunks 128/128/66) into PSUM
    [128, 2048] (4 banks), one Exp activation over the 4 banks with
    accum_out producing the per-partition block sum
  - out "acc" [128, 17] fp32
"""

import os

import numpy as np

import concourse.bacc as bacc
import concourse.bass as bass
import concourse.mybir as mybir
import concourse.tile as tile
from concourse.bass_utils import run_bass_kernel_spmd

B = 4096
D0, D1 = 256, 64
N = 2 * B
CH = 512          # rows per chunk
NCHUNK = 16
NCORE = 8
KF = D0 + D1      # feature contraction rows
KT = KF + 2       # + ones row + (-c) row
KQ = [(0, 128), (128, 256), (256, KT)]   # contraction partition chunks
MT = 128          # m-tile rows
NMT = CH // MT    # m-tiles per row-chunk (4)
NBLK = 17         # blocks per core (9 from chunk k, 8 from chunk k+8)
NCOL = NBLK       # acc columns

# bf16 streams the PE at 1 cyc/row (f32r: 2, f32: 4); measured end loss
# error vs float64 is ~1.5e-4 rel — far inside the fp32-envelope budget.
_DT_NAME = os.environ.get("JMMD_MM_DTYPE", "bf16")
_DT = {
    "f32r": mybir.dt.float32r,
    "f32": mybir.dt.float32,
    "bf16": mybir.dt.bfloat16,
}[_DT_NAME]
_N_WARMUP = int(os.environ.get("JMMD_WARMUP", "28"))

LAST_EXEC_NS = None
LAST_RESULTS = None

_CACHE: dict = {}


def _np_dtype():
    if _DT_NAME == "bf16":
        import ml_dtypes

        return ml_dtypes.bfloat16
    return np.float32


def _build():
    if "nc" in _CACHE:
        return _CACHE["nc"]
    nc = bacc.Bacc(
        "TRN2", target_bir_lowering=False, debug=False, enable_asserts=False
    )
    f32 = mybir.dt.float32
    lhs_dram = nc.dram_tensor("lhs", [2, KT, CH], _DT, kind="ExternalInput").ap()
    rhs_dram = nc.dram_tensor("rhs", [NCHUNK, KT, CH], _DT, kind="ExternalInput").ap()
    acc_dram = nc.dram_tensor("acc", [MT, NCOL], f32, kind="ExternalOutput").ap()

    with tile.TileContext(nc) as tc:
        with (
            tc.tile_pool(name="const", bufs=1) as const,
            tc.tile_pool(name="psum", bufs=2, space=bass.MemorySpace.PSUM) as psum,
        ):
            lhs_t = {}
            rhs_t = {}

            def load_lhs(g):
                for q, (a, b) in enumerate(KQ):
                    t = const.tile([b - a, CH], _DT, tag=f"lhs{g}_{q}")
                    nc.sync.dma_start(t[:], lhs_dram[g, a:b, :])
                    lhs_t[(g, q)] = t

            def load_rhs(ch, eng):
                for q, (a, b) in enumerate(KQ):
                    t = const.tile([b - a, CH], _DT, tag=f"rhs{q}_{ch}")
                    eng.dma_start(t[:], rhs_dram[ch, a:b, :])
                    rhs_t[(q, ch)] = t

            # warmup scratch memset goes FIRST on gpsimd — anything queued
            # behind bulk DMAs on that engine would stall the PE program.
            scratch = None
            if _N_WARMUP:
                scratch = const.tile([MT, 256], _DT, tag="warm_src")
                nc.gpsimd.memset(scratch[:], 0.0)

            # block 0's operands race down both DMA engines in parallel;
            # lhsB is not needed until block 9 (~t+23us)
            load_lhs(0)
            load_rhs(0, nc.gpsimd)
            for ch in (1, 3, 5):
                load_rhs(ch, nc.sync)
            for ch in (2, 4, 6):
                load_rhs(ch, nc.gpsimd)
            load_lhs(1)
            for ch in (7, 9, 11, 13, 15):
                load_rhs(ch, nc.sync)
            for ch in (8, 10, 12, 14):
                load_rhs(ch, nc.gpsimd)

            acc_t = const.tile([MT, NCOL], f32, tag="acc")

            # HAM warmup: dense dummy matmuls while input DMAs stream, so
            # real matmuls start at the warm PE clock.
            if _N_WARMUP:
                warm_ps = psum.tile([MT, NMT * CH], f32, tag="ps")
                for _ in range(_N_WARMUP):
                    nc.tensor.matmul(
                        warm_ps[:, :MT],
                        scratch[:, :MT],
                        scratch[:, MT:],
                        start=True,
                        stop=True,
                    )

            for g, nd in ((0, 9), (1, 8)):
                for d in range(nd):
                    ch = d if g == 0 else 8 + d
                    col = d if g == 0 else 9 + d
                    ps = psum.tile([MT, NMT * CH], f32, tag="ps")
                    for m in range(NMT):
                        for q in range(3):
                            nc.tensor.matmul(
                                ps[:, m * CH:(m + 1) * CH],
                                lhs_t[(g, q)][:, m * MT:(m + 1) * MT],
                                rhs_t[(q, ch)][:],
                                start=(q == 0),
                                stop=(q == 2),
                            )
                    nc.scalar.activation(
                        ps[:],
                        ps[:],
                        mybir.ActivationFunctionType.Exp,
                        accum_out=acc_t[:, col:col + 1],
                    )
            nc.sync.dma_start(acc_dram[:], acc_t[:])
    nc.compile()
    _CACHE["nc"] = nc
    return nc


def _pack_inputs(s0, s1, t0, t1):
    X0 = np.concatenate([s0, t0], axis=0).astype(np.float64)
    X1 = np.concatenate([s1, t1], axis=0).astype(np.float64)

    def gamma_of(X):
        sq = np.sum(X * X, axis=1)
        sdist = 2.0 * X.shape[0] * np.sum(sq) - 2.0 * np.sum(np.sum(X, axis=0) ** 2)
        return (X.shape[0] ** 2 - X.shape[0]) / sdist, sq

    g0, sq0 = gamma_of(X0)
    g1, sq1 = gamma_of(X1)
    c = g0 * sq0 + g1 * sq1
    W = np.concatenate([np.sqrt(g0) * X0, np.sqrt(g1) * X1], axis=1)  # [N, 320]
    npdt = _np_dtype()

    # chunk-major staging of [W.T ; -c ; ones] so every device DMA reads a
    # contiguous range
    Wt = np.empty((NCHUNK, KT, CH), dtype=np.float64)
    for ch in range(NCHUNK):
        rows = slice(ch * CH, (ch + 1) * CH)
        Wt[ch, :KF] = W[rows].T
        Wt[ch, KF] = -c[rows]
        Wt[ch, KF + 1] = 1.0
    Wt = Wt.astype(npdt)

    def lhs_for(chunk):
        rows = slice(chunk * CH, (chunk + 1) * CH)
        out = np.empty((KT, CH), dtype=np.float64)
        out[:KF] = 2.0 * W[rows].T
        out[KF] = 1.0
        out[KF + 1] = -c[rows]
        return out.astype(npdt)

    in_maps = []
    for k in range(NCORE):
        lhs = np.stack([lhs_for(k), lhs_for((k + 8) % NCHUNK)])
        rhs = Wt[[(k + d) % NCHUNK for d in range(NCHUNK)]]
        in_maps.append({"lhs": lhs, "rhs": np.ascontiguousarray(rhs)})
    return in_maps


def _combine(results):
    sgn = lambda ch: 1.0 if ch < NCHUNK // 2 else -1.0
    total = 0.0
    for k in range(NCORE):
        acc = np.asarray(results[k]["acc"], dtype=np.float64)  # [128, 17]
        colsum = acc.sum(axis=0)
        for col in range(NCOL):
            if col < 9:
                d, row_chunk = col, k
            else:
                d, row_chunk = col - 9, (k + 8) % NCHUNK
            col_chunk = (row_chunk + d) % NCHUNK
            w = 1.0 if d == 0 else 2.0
            s = sgn(row_chunk) * sgn(col_chunk)
            total += w * s * colsum[col]
    return total / (B * B)


def kernel(s0, s1, t0, t1):
    global LAST_EXEC_NS, LAST_RESULTS
    nc = _build()
    in_maps = _pack_inputs(
        np.asarray(s0), np.asarray(s1), np.asarray(t0), np.asarray(t1)
    )
    trace = os.environ.get("JMMD_TRACE", "0") == "1"
    res = run_bass_kernel_spmd(nc, in_maps, core_ids=list(range(NCORE)), trace=trace)
    LAST_EXEC_NS = res.exec_time_ns
    LAST_RESULTS = res
    return np.float32(_combine(res.results))



# revision 3
# speedup vs baseline: 1.0615x; 1.0615x over previous
"""Joint Maximum Mean Discrepancy loss on 8 Trainium2 NeuronCores.

Math: for streams (s0,t0) and (s1,t1), the reference builds per-stream
Gaussian kernels K_r = exp(-gamma_r * dist_r) over feats_r = [src; tgt]
(N=8192 rows), takes their elementwise product, and returns
mean(s2s + t2t - 2*s2t) over the B x B blocks.

Device decomposition:
  exponent E_ij = 2*g0*(X0_i . X0_j) + 2*g1*(X1_i . X1_j) - c_i - c_j,
  c_i = g0*|X0_i|^2 + g1*|X1_i|^2, gamma_r from the closed form
  sum(dist_r) = 2*N*sum(sq_r) - 2*||colsum(X_r)||^2. The joint kernel is
  exp(E); the loss is a signed/weighted sum of exp over the 136 unordered
  512-row chunk-pair blocks (symmetry halves the N x N work).

  PSUM accumulates P = SCALE*E from two matmuls per [128,512] m-tile:
    - fp8 e4m3 DoubleRow over the 256 stream-0 rows (2 K-rows/partition,
      2x PE rate): rows sqrt(2*g0*SCALE)*X0, layout [128, 2, 512] with
      tile[p,s,x] = W0T[s*128+p, x]
    - bf16 over 66 rows: [sqrt(2*g1*SCALE)*X1 (64) ; ones ; -SCALE*c]
      (lhs variant) vs [... ; -SCALE*c ; ones] (rhs variant)
  ScalarE applies Exp with scale=1/SCALE into SBUF bf16; VectorE
  tensor-reduces each block to a per-partition sum column (acc [128,18]).
  fp8 end-to-end loss error vs float64 is ~2e-3 (measured host-sim),
  well inside the 2e-2 budget.

Block cover (SPMD): a fixed 18-block pattern over 8 chunk "slots";
core k maps slot v to chunk (S[v] + 2k) mod 16, S = (0,1,2,3,4,5,8,9).
The 8 shifted copies tile all 120 chunk pairs + 16 loops: difference
classes d=1..7 x base-parity are hit exactly once (host weight 2),
d=8 pairs twice (weight 1), loops once (weight 1). The host applies
weight * sign (sign -1 iff exactly one chunk is a target chunk >= 8)
and reduces in float64.
"""

import os

import numpy as np

import concourse.bacc as bacc
import concourse.bass as bass
import concourse.mybir as mybir
import concourse.tile as tile
from concourse.bass_utils import run_bass_kernel_spmd

B = 4096
D0, D1 = 256, 64
N = 2 * B
CH = 512          # rows per chunk
NCHUNK = 16
NCORE = 8
MT = 128          # m-tile rows
NMT = CH // MT    # m-tiles per block row (4)
SCALE = 64.0      # exponent pre-scale; exp applies 1/SCALE

# cyclic support: slot v of core k is chunk (S[v] + 2k) % 16
S_SUPPORT = (0, 1, 2, 3, 4, 5, 8, 9)
NSLOT = 8
# 18-block pattern in slot indices, ordered so early blocks touch early
# slots (DMA pipelining): loops (0,0),(1,1); one pair per (diff 1..7,
# parity) class; both d=8 classes.
PATTERN = [
    (0, 0), (1, 1), (0, 1),
    (1, 2), (0, 2),
    (1, 3), (0, 3),
    (1, 4), (0, 4),
    (1, 5), (0, 5),
    (3, 6), (2, 6), (1, 6), (0, 6),
    (3, 7), (2, 7), (1, 7),
]
NBLK = len(PATTERN)  # 18

F8 = mybir.dt.float8e4
BF = mybir.dt.bfloat16
F32 = mybir.dt.float32
KB = D1 + 2       # bf16 contraction rows (66)

_N_WARMUP = int(os.environ.get("JMMD_WARMUP", "28"))

LAST_EXEC_NS = None
LAST_RESULTS = None

_CACHE: dict = {}


def _build():
    if "nc" in _CACHE:
        return _CACHE["nc"]
    nc = bacc.Bacc(
        "TRN2", target_bir_lowering=False, debug=False, enable_asserts=False
    )
    f8_dram = nc.dram_tensor("f8", [NSLOT, MT, 2, CH], F8, kind="ExternalInput").ap()
    lb_dram = nc.dram_tensor("lb", [NSLOT, KB, CH], BF, kind="ExternalInput").ap()
    rb_dram = nc.dram_tensor("rb", [NSLOT, KB, CH], BF, kind="ExternalInput").ap()
    acc_dram = nc.dram_tensor("acc", [MT, NBLK], F32, kind="ExternalOutput").ap()

    with tile.TileContext(nc) as tc:
        with (
            tc.tile_pool(name="const", bufs=1) as const,
            tc.tile_pool(name="exp", bufs=2) as expp,
            tc.tile_pool(name="psum", bufs=2, space=bass.MemorySpace.PSUM) as psum,
        ):
            # warmup scratch memset goes FIRST on gpsimd — anything queued
            # behind bulk DMAs on that engine would stall the PE program.
            scratch = None
            if _N_WARMUP:
                scratch = const.tile([MT, 256], BF, tag="warm_src")
                nc.gpsimd.memset(scratch[:], 0.0)

            ft, lt, rt = {}, {}, {}
            for j in range(NSLOT):
                ft[j] = const.tile([MT, 2, CH], F8, name=f"f{j}", tag=f"f{j}")
                lt[j] = const.tile([KB, CH], BF, name=f"l{j}", tag=f"l{j}")
                rt[j] = const.tile([KB, CH], BF, name=f"r{j}", tag=f"r{j}")
                qa, qb = (nc.sync, nc.gpsimd) if j % 2 == 0 else (nc.gpsimd, nc.sync)
                qa.dma_start(ft[j][:], f8_dram[j])
                qb.dma_start(lt[j][:], lb_dram[j])
                qb.dma_start(rt[j][:], rb_dram[j])

            acc_t = const.tile([MT, NBLK], F32, tag="acc")

            if _N_WARMUP:
                # ACT table preload for Exp while input DMAs stream
                warm_act = const.tile([MT, 8], BF, tag="warm_act")
                nc.scalar.activation(
                    warm_act[:], scratch[:, :8], mybir.ActivationFunctionType.Exp
                )
                # HAM warmup: dummy matmuls so real matmuls start warm
                warm_ps = psum.tile([MT, NMT * CH], F32, tag="ps")
                for _ in range(_N_WARMUP):
                    nc.tensor.matmul(
                        warm_ps[:, :MT],
                        scratch[:, :MT],
                        scratch[:, MT:],
                        start=True,
                        stop=True,
                    )

            for col, (r, c) in enumerate(PATTERN):
                ps = psum.tile([MT, NMT * CH], F32, tag="ps")
                for m in range(NMT):
                    nc.tensor.matmul(
                        ps[:, m * CH:(m + 1) * CH],
                        ft[r][:, :, m * MT:(m + 1) * MT],
                        ft[c][:],
                        start=True,
                        stop=False,
                        perf_mode=mybir.MatmulPerfMode.DoubleRow,
                    )
                    nc.tensor.matmul(
                        ps[:, m * CH:(m + 1) * CH],
                        lt[r][:, m * MT:(m + 1) * MT],
                        rt[c][:],
                        start=False,
                        stop=True,
                    )
                ex = expp.tile([MT, NMT * CH], BF, tag="ex")
                nc.scalar.activation(
                    ex[:], ps[:], mybir.ActivationFunctionType.Exp, scale=1.0 / SCALE
                )
                nc.vector.tensor_reduce(
                    acc_t[:, col:col + 1],
                    ex[:],
                    axis=mybir.AxisListType.X,
                    op=mybir.AluOpType.add,
                )
            nc.sync.dma_start(acc_dram, acc_t[:])
    nc.compile()
    _CACHE["nc"] = nc
    return nc


def _pack_inputs(s0, s1, t0, t1):
    import ml_dtypes

    X0 = np.concatenate([s0, t0], axis=0).astype(np.float64)
    X1 = np.concatenate([s1, t1], axis=0).astype(np.float64)

    def gamma_of(X):
        sq = np.sum(X * X, axis=1)
        sdist = 2.0 * N * np.sum(sq) - 2.0 * np.sum(np.sum(X, axis=0) ** 2)
        return (N * N - N) / sdist, sq

    g0, sq0 = gamma_of(X0)
    g1, sq1 = gamma_of(X1)
    c = g0 * sq0 + g1 * sq1

    W0 = np.sqrt(2.0 * g0 * SCALE) * X0          # [N, 256] -> fp8
    W1 = np.sqrt(2.0 * g1 * SCALE) * X1          # [N, 64]  -> bf16
    W0q = np.clip(W0, -240, 240).astype(ml_dtypes.float8_e4m3)
    W1q = W1.astype(ml_dtypes.bfloat16)
    cq = (-SCALE * c).astype(ml_dtypes.bfloat16)

    # per-chunk staged tiles
    fch = []
    lch = []
    rch = []
    for ch in range(NCHUNK):
        rows = slice(ch * CH, (ch + 1) * CH)
        A = W0q[rows].T                          # [256, 512]
        fch.append(np.ascontiguousarray(A.reshape(2, MT, CH).transpose(1, 0, 2)))
        lb = np.empty((KB, CH), dtype=ml_dtypes.bfloat16)
        rb = np.empty((KB, CH), dtype=ml_dtypes.bfloat16)
        lb[:D1] = W1q[rows].T
        rb[:D1] = W1q[rows].T
        lb[D1] = 1.0
        lb[D1 + 1] = cq[rows]
        rb[D1] = cq[rows]
        rb[D1 + 1] = 1.0
        lch.append(lb)
        rch.append(rb)

    in_maps = []
    for k in range(NCORE):
        slots = [(S_SUPPORT[v] + 2 * k) % NCHUNK for v in range(NSLOT)]
        in_maps.append(
            {
                "f8": np.ascontiguousarray(np.stack([fch[ch] for ch in slots])),
                "lb": np.ascontiguousarray(np.stack([lch[ch] for ch in slots])),
                "rb": np.ascontiguousarray(np.stack([rch[ch] for ch in slots])),
            }
        )
    return in_maps


def _combine(results):
    total = 0.0
    for k in range(NCORE):
        acc = np.asarray(results[k]["acc"], dtype=np.float64)  # [128, NBLK]
        colsum = acc.sum(axis=0)
        for col, (r, c) in enumerate(PATTERN):
            u = (S_SUPPORT[r] + 2 * k) % NCHUNK
            v = (S_SUPPORT[c] + 2 * k) % NCHUNK
            d = min((v - u) % NCHUNK, (u - v) % NCHUNK)
            w = 2.0 if 0 < d < 8 else 1.0        # loops and d=8 (doubled): 1
            s = (1.0 if u < 8 else -1.0) * (1.0 if v < 8 else -1.0)
            total += w * s * colsum[col]
    return total / (B * B)


def kernel(s0, s1, t0, t1):
    global LAST_EXEC_NS, LAST_RESULTS
    nc = _build()
    in_maps = _pack_inputs(
        np.asarray(s0), np.asarray(s1), np.asarray(t0), np.asarray(t1)
    )
    trace = os.environ.get("JMMD_TRACE", "0") == "1"
    res = run_bass_kernel_spmd(nc, in_maps, core_ids=list(range(NCORE)), trace=trace)
    LAST_EXEC_NS = res.exec_time_ns
    LAST_RESULTS = res
    return np.float32(_combine(res.results))
